# revision 5
# baseline (speedup 1.0000x reference)
"""Distributed brute-force KNN (IndexFlatL2, K=3) + mean of gathered pred values.

Strategy (data-parallel over the memory bank N, queries replicated):
  - Each of the 8 cores gets a shard of 12500 memory rows (transposed on host
    so the PE moving operand [K=d, N=n] streams directly from DRAM).
  - Device computes s'[b, n] = 2*q.b - ||m_n||^2 per shard with fp32 matmuls
    (ranking by largest s' == smallest L2 distance; q_sq is row-constant and
    drops out of the ranking).
  - Per 500-column block, DVE max8 + max_index produce the block's top-8
    values and indices -> 200 candidates per query per core.
  - Host merges 8 cores x 200 candidates, takes the global top-3 per query,
    gathers pred_values, and returns the scalar mean.
"""

import sys
import types

import numpy as np

try:  # bass_utils' axon trace path imports this unconditionally when
    import antenv.axon_hooks  # noqa: F401  # BASS_TRACE is set; stub it if absent
except ImportError:
    _stub = types.ModuleType("antenv.axon_hooks")
    _stub.get_axon_ntff_profile_hook = lambda: None
    _stub.set_axon_ntff_profile_hook = lambda hook: None
    sys.modules["antenv.axon_hooks"] = _stub

import concourse.bacc as bacc
import concourse.mybir as mybir
import concourse.tile as tile
from concourse import bass_utils

B = 1024            # queries
D = 1024            # embedding dim
N = 100000          # memory rows
NCORES = 8
NS = N // NCORES    # 12500 memory rows per core
BLK = 500           # matmul free-dim tile (fits one PSUM bank in fp32)
NBLK = NS // BLK    # 25 blocks per core
GROUP_W = 3         # blocks fetched per DMA group
KT = D // 128       # 8 contraction tiles
BCH = B // 128      # 8 query chunks of 128
TOPB = 8            # DVE max8 width
CAND = NBLK * TOPB  # 200 candidates per query per core
K = 3

_CACHE = {}
LAST_RUN = None
LAST_TOP_IDX = None


def _build_program(nblk=NBLK, bch=BCH, group_w=GROUP_W):
    nc = bacc.Bacc(
        "TRN2",
        target_bir_lowering=False,
        debug=False,
        enable_asserts=False,
        num_devices=NCORES,
    )
    f32 = mybir.dt.float32
    u32 = mybir.dt.uint32
    ns = nblk * BLK
    cand = nblk * TOPB
    b = bch * 128

    mT = nc.dram_tensor("mT", [D, ns], f32, kind="ExternalInput").ap()
    qT = nc.dram_tensor("qT", [D, b], f32, kind="ExternalInput").ap()
    msq = nc.dram_tensor("msq", [1, ns], f32, kind="ExternalInput").ap()
    out_vals = nc.dram_tensor("out_vals", [b, cand], f32, kind="ExternalOutput").ap()
    out_idx = nc.dram_tensor("out_idx", [b, cand], u32, kind="ExternalOutput").ap()

    mT_r = mT.rearrange("(o p) n -> p o n", p=128)
    qT_r = qT.rearrange("(o p) b -> p o b", p=128)
    ov_r = out_vals.rearrange("(c p) j -> p c j", p=128)
    oi_r = out_idx.rearrange("(c p) j -> p c j", p=128)

    groups = []
    g0 = 0
    while g0 < nblk:
        w = min(group_w, nblk - g0)
        groups.append((g0, w))
        g0 += w

    with tile.TileContext(nc) as tc:
        with (
            tc.tile_pool(name="const", bufs=1) as cpool,
            tc.tile_pool(name="mov", bufs=2) as movpool,
            tc.tile_pool(name="msqp", bufs=2) as msqpool,
            tc.tile_pool(name="score", bufs=6) as scpool,
            tc.tile_pool(name="psum", bufs=6, space="PSUM") as pspool,
        ):
            qt_sb = cpool.tile([128, KT, b], f32, tag="qt")
            nc.sync.dma_start(qt_sb, qT_r)
            cand_v = cpool.tile([128, bch, cand], f32, tag="cv")
            cand_i = cpool.tile([128, bch, cand], u32, tag="ci")

            for blk0, w in groups:
                n0 = blk0 * BLK
                wn = w * BLK
                mov = movpool.tile([128, KT, group_w * BLK], f32, tag="mov")
                nc.sync.dma_start(mov[:, :, :wn], mT_r[:, :, n0 : n0 + wn])
                msqb = msqpool.tile([128, group_w * BLK], f32, tag="msqb")
                nc.sync.dma_start(
                    msqb[:, :wn], msq[:, n0 : n0 + wn].to_broadcast([128, wn])
                )
                for bc in range(bch):
                    psums = [
                        pspool.tile([128, BLK], f32, tag="mm", name="mm_ps") for _ in range(w)
                    ]
                    for k in range(KT):
                        lhsT = qt_sb[:, k, bc * 128 : (bc + 1) * 128]
                        for j in range(w):
                            nc.tensor.matmul(
                                psums[j],
                                lhsT=lhsT,
                                rhs=mov[:, k, j * BLK : (j + 1) * BLK],
                                start=(k == 0),
                                stop=(k == KT - 1),
                            )
                    for j in range(w):
                        blk = blk0 + j
                        sc = scpool.tile([128, BLK], f32, tag="sc", name="sc")
                        nc.vector.tensor_sub(
                            sc, psums[j], msqb[:, j * BLK : (j + 1) * BLK]
                        )
                        nc.vector.max(
                            out=cand_v[:, bc, blk * TOPB : (blk + 1) * TOPB], in_=sc
                        )
                        nc.vector.max_index(
                            out=cand_i[:, bc, blk * TOPB : (blk + 1) * TOPB],
                            in_max=cand_v[:, bc, blk * TOPB : (blk + 1) * TOPB],
                            in_values=sc,
                        )
            nc.sync.dma_start(ov_r, cand_v)
            nc.sync.dma_start(oi_r, cand_i)
    nc.compile()
    return nc


def kernel(h_query, memory_embeds, pred_values):
    global LAST_RUN
    q = np.ascontiguousarray(np.asarray(h_query, dtype=np.float32))
    m = np.ascontiguousarray(np.asarray(memory_embeds, dtype=np.float32))
    pv = np.asarray(pred_values, dtype=np.float32)

    qT2 = np.ascontiguousarray(q.T) * np.float32(2.0)
    msq = np.einsum("nd,nd->n", m, m)
    mT = m.T

    if "nc" not in _CACHE:
        _CACHE["nc"] = _build_program()
    nc = _CACHE["nc"]

    in_maps = []
    for c in range(NCORES):
        sl = slice(c * NS, (c + 1) * NS)
        in_maps.append(
            {
                "mT": np.ascontiguousarray(mT[:, sl]),
                "qT": qT2,
                "msq": np.ascontiguousarray(msq[sl]).reshape(1, NS),
            }
        )

    res = bass_utils.run_bass_kernel_spmd(nc, in_maps, core_ids=list(range(NCORES)))
    LAST_RUN = res
    results = res.results

    vals = np.concatenate([r["out_vals"] for r in results], axis=1)
    blk_off = (np.arange(CAND, dtype=np.int64) // TOPB) * BLK
    idxs = np.concatenate(
        [
            r["out_idx"].astype(np.int64) + blk_off[None, :] + c * NS
            for c, r in enumerate(results)
        ],
        axis=1,
    )
    sel = np.argpartition(-vals, K, axis=1)[:, :K]
    top_idx = np.take_along_axis(idxs, sel, axis=1)
    global LAST_TOP_IDX
    LAST_TOP_IDX = top_idx
    y = pv[top_idx].astype(np.float64).mean()
    return np.float32(y)


# revision 6
# speedup vs baseline: 1.5560x; 1.5560x over previous
"""Distributed brute-force KNN (IndexFlatL2, K=3) + mean of gathered pred values.

Strategy (data-parallel over the memory bank N, queries replicated):
  - Each of the 8 cores gets a shard of 12500 memory rows (transposed on host
    so the PE moving operand [K=d, N=n] streams directly from DRAM).
  - Scores s'[b, n] = 2*q.m_n - ||m_n||^2 (ranking by largest s' == smallest
    L2 distance; q_sq is row-constant and drops out of the ranking).
  - The matmul runs as an fp16 hi/lo split: x = hi + lo with hi = fp16(x),
    lo = fp16(x - hi); c = q_hi.m_hi + q_hi.m_lo + q_lo.m_hi accumulated in
    fp32 PSUM. That's ~22 effective mantissa bits (score error ~1e-5, far
    below typical 3rd/4th-neighbor distance gaps ~O(1)) at 3 fp16 passes =
    3 PE cycles/column instead of fp32's effective ~6.7.
  - Per 500-column block, DVE max8 + max_index produce the block's top-8
    values and indices -> 200 candidates per query per core.
  - Host merges 8 cores x 200 candidates, takes the global top-3 per query,
    gathers pred_values, and returns the scalar mean.
"""

import sys
import types

import numpy as np

try:  # bass_utils' axon trace path imports this unconditionally when
    import antenv.axon_hooks  # noqa: F401  # BASS_TRACE is set; stub it if absent
except ImportError:
    _stub = types.ModuleType("antenv.axon_hooks")
    _stub.get_axon_ntff_profile_hook = lambda: None
    _stub.set_axon_ntff_profile_hook = lambda hook: None
    sys.modules["antenv.axon_hooks"] = _stub

import concourse.bacc as bacc
import concourse.mybir as mybir
import concourse.tile as tile
from concourse import bass_utils

B = 1024            # queries
D = 1024            # embedding dim
N = 100000          # memory rows
NCORES = 8
NS = N // NCORES    # 12500 memory rows per core
BLK = 500           # matmul free-dim tile (fits one PSUM bank in fp32)
NBLK = NS // BLK    # 25 blocks per core
GROUP_W = 3         # blocks fetched per DMA group
KT = D // 128       # 8 contraction tiles
BCH = B // 128      # 8 query chunks of 128
TOPB = 8            # DVE max8 width
CAND = NBLK * TOPB  # 200 candidates per query per core
K = 3

_CACHE = {}
LAST_RUN = None
LAST_TOP_IDX = None


def _build_program(nblk=NBLK, bch=BCH, group_w=GROUP_W):
    nc = bacc.Bacc(
        "TRN2",
        target_bir_lowering=False,
        debug=False,
        enable_asserts=False,
        num_devices=NCORES,
    )
    f32 = mybir.dt.float32
    f16 = mybir.dt.float16
    u32 = mybir.dt.uint32
    ns = nblk * BLK
    cand = nblk * TOPB
    b = bch * 128

    mT_hi = nc.dram_tensor("mT_hi", [D, ns], f16, kind="ExternalInput").ap()
    mT_lo = nc.dram_tensor("mT_lo", [D, ns], f16, kind="ExternalInput").ap()
    qT_hi = nc.dram_tensor("qT_hi", [D, b], f16, kind="ExternalInput").ap()
    qT_lo = nc.dram_tensor("qT_lo", [D, b], f16, kind="ExternalInput").ap()
    msq = nc.dram_tensor("msq", [1, ns], f32, kind="ExternalInput").ap()
    out_vals = nc.dram_tensor("out_vals", [b, cand], f32, kind="ExternalOutput").ap()
    out_idx = nc.dram_tensor("out_idx", [b, cand], u32, kind="ExternalOutput").ap()

    mhi_r = mT_hi.rearrange("(o p) n -> p o n", p=128)
    mlo_r = mT_lo.rearrange("(o p) n -> p o n", p=128)
    qhi_r = qT_hi.rearrange("(o p) b -> p o b", p=128)
    qlo_r = qT_lo.rearrange("(o p) b -> p o b", p=128)
    ov_r = out_vals.rearrange("(c p) j -> p c j", p=128)
    oi_r = out_idx.rearrange("(c p) j -> p c j", p=128)

    groups = []
    g0 = 0
    while g0 < nblk:
        w = min(group_w, nblk - g0)
        groups.append((g0, w))
        g0 += w

    with tile.TileContext(nc) as tc:
        with (
            tc.tile_pool(name="const", bufs=1) as cpool,
            tc.tile_pool(name="mov", bufs=2) as movpool,
            tc.tile_pool(name="msqp", bufs=2) as msqpool,
            tc.tile_pool(name="score", bufs=6) as scpool,
            tc.tile_pool(name="psum", bufs=8, space="PSUM") as pspool,
        ):
            qt_hi = cpool.tile([128, KT, b], f16, tag="qthi")
            nc.sync.dma_start(qt_hi, qhi_r)
            qt_lo = cpool.tile([128, KT, b], f16, tag="qtlo")
            nc.sync.dma_start(qt_lo, qlo_r)
            cand_v = cpool.tile([128, bch, cand], f32, tag="cv")
            cand_i = cpool.tile([128, bch, cand], u32, tag="ci")

            for blk0, w in groups:
                n0 = blk0 * BLK
                wn = w * BLK
                mov_hi = movpool.tile([128, KT, group_w * BLK], f16, tag="movhi")
                nc.sync.dma_start(mov_hi[:, :, :wn], mhi_r[:, :, n0 : n0 + wn])
                mov_lo = movpool.tile([128, KT, group_w * BLK], f16, tag="movlo")
                nc.sync.dma_start(mov_lo[:, :, :wn], mlo_r[:, :, n0 : n0 + wn])
                msqb = msqpool.tile([128, group_w * BLK], f32, tag="msqb")
                nc.sync.dma_start(
                    msqb[:, :wn], msq[:, n0 : n0 + wn].to_broadcast([128, wn])
                )
                for bc in range(bch):
                    psums = [
                        pspool.tile([128, BLK], f32, tag="mm", name="mm_ps")
                        for _ in range(w)
                    ]
                    # 3-term fp16 split: (q_hi, m_hi), (q_hi, m_lo), (q_lo, m_hi).
                    # Combos grouped by lhsT so consecutive matmuls share weights.
                    combos = [(qt_hi, mov_hi), (qt_hi, mov_lo), (qt_lo, mov_hi)]
                    for k in range(KT):
                        for ci, (qs, ms) in enumerate(combos):
                            lhsT = qs[:, k, bc * 128 : (bc + 1) * 128]
                            for j in range(w):
                                nc.tensor.matmul(
                                    psums[j],
                                    lhsT=lhsT,
                                    rhs=ms[:, k, j * BLK : (j + 1) * BLK],
                                    start=(k == 0 and ci == 0),
                                    stop=(k == KT - 1 and ci == len(combos) - 1),
                                )
                    for j in range(w):
                        blk = blk0 + j
                        sc = scpool.tile([128, BLK], f32, tag="sc", name="sc")
                        nc.vector.tensor_sub(
                            sc, psums[j], msqb[:, j * BLK : (j + 1) * BLK]
                        )
                        nc.vector.max(
                            out=cand_v[:, bc, blk * TOPB : (blk + 1) * TOPB], in_=sc
                        )
                        nc.vector.max_index(
                            out=cand_i[:, bc, blk * TOPB : (blk + 1) * TOPB],
                            in_max=cand_v[:, bc, blk * TOPB : (blk + 1) * TOPB],
                            in_values=sc,
                        )
            nc.sync.dma_start(ov_r, cand_v)
            nc.sync.dma_start(oi_r, cand_i)
    nc.compile()
    return nc


def _split_f16(x):
    hi = x.astype(np.float16)
    lo = (x - hi.astype(np.float32)).astype(np.float16)
    return hi, lo


def kernel(h_query, memory_embeds, pred_values):
    global LAST_RUN, LAST_TOP_IDX
    q = np.ascontiguousarray(np.asarray(h_query, dtype=np.float32))
    m = np.ascontiguousarray(np.asarray(memory_embeds, dtype=np.float32))
    pv = np.asarray(pred_values, dtype=np.float32)

    qT2 = np.ascontiguousarray(q.T) * np.float32(2.0)
    q_hi, q_lo = _split_f16(qT2)
    msq = np.einsum("nd,nd->n", m, m)
    mT = np.ascontiguousarray(m.T)
    m_hi, m_lo = _split_f16(mT)

    if "nc" not in _CACHE:
        _CACHE["nc"] = _build_program()
    nc = _CACHE["nc"]

    in_maps = []
    for c in range(NCORES):
        sl = slice(c * NS, (c + 1) * NS)
        in_maps.append(
            {
                "mT_hi": np.ascontiguousarray(m_hi[:, sl]),
                "mT_lo": np.ascontiguousarray(m_lo[:, sl]),
                "qT_hi": q_hi,
                "qT_lo": q_lo,
                "msq": np.ascontiguousarray(msq[sl]).reshape(1, NS),
            }
        )

    res = bass_utils.run_bass_kernel_spmd(nc, in_maps, core_ids=list(range(NCORES)))
    LAST_RUN = res
    results = res.results

    vals = np.concatenate([r["out_vals"] for r in results], axis=1)
    blk_off = (np.arange(CAND, dtype=np.int64) // TOPB) * BLK
    idxs = np.concatenate(
        [
            r["out_idx"].astype(np.int64) + blk_off[None, :] + c * NS
            for c, r in enumerate(results)
        ],
        axis=1,
    )
    sel = np.argpartition(-vals, K, axis=1)[:, :K]
    top_idx = np.take_along_axis(idxs, sel, axis=1)
    LAST_TOP_IDX = top_idx
    y = pv[top_idx].astype(np.float64).mean()
    return np.float32(y)


# revision 8
# speedup vs baseline: 4.0384x; 2.5954x over previous
"""Distributed brute-force KNN (IndexFlatL2, K=3) + mean of gathered pred values.

Strategy (data-parallel over the memory bank N, queries replicated):
  - Each of the 8 cores gets a shard of 12500 memory rows (transposed on host
    so the PE moving operand [K=d, N=n] streams directly from DRAM).
  - Phase 1 (device): approximate scores s'[b, n] = 2*q.m_n - ||m_n||^2
    (ranking by largest s' == smallest L2 distance; q_sq is row-constant and
    drops out). One fp16 x fp16 matmul pass (1 PE cycle/column), fp32 PSUM
    accumulation, msq subtracted exactly on DVE during the PSUM drain.
    Score noise ~0.02 + fp16 score quantization ~0.5 is far below the ~20+
    value gap down to a block's rank-8, so the true top-3 always survive
    into the candidate set. Per 500-column block, DVE max8 + max_index (in
    16-bit 2x/4x modes) return the block's top-8 values and indices
    -> 200 candidates per query per core.
  - Phase 2 (host): merge 8 cores x 200 candidates, exactly re-score the
    top-24 approximate candidates per query in fp64, take the true top-3,
    gather pred_values, return the scalar mean.
"""

import sys
import types

import numpy as np

try:  # bass_utils' axon trace path imports this unconditionally when
    import antenv.axon_hooks  # noqa: F401  # BASS_TRACE is set; stub it if absent
except ImportError:
    _stub = types.ModuleType("antenv.axon_hooks")
    _stub.get_axon_ntff_profile_hook = lambda: None
    _stub.set_axon_ntff_profile_hook = lambda hook: None
    sys.modules["antenv.axon_hooks"] = _stub

import concourse.bacc as bacc
import concourse.mybir as mybir
import concourse.tile as tile
from concourse import bass_utils

B = 1024            # queries
D = 1024            # embedding dim
N = 100000          # memory rows
NCORES = 8
NS = N // NCORES    # 12500 memory rows per core
BLK = 500           # matmul free-dim tile (fits one PSUM bank in fp32)
NBLK = NS // BLK    # 25 blocks per core
GROUP_W = 5         # blocks fetched per DMA group (25 = 5 uniform groups)
KT = D // 128       # 8 contraction tiles
BCH = B // 128      # 8 query chunks of 128
TOPB = 8            # DVE max8 width
CAND = NBLK * TOPB  # 200 candidates per query per core
K = 3
RERANK = 24         # candidates exactly re-scored on host per query

_CACHE = {}
LAST_RUN = None
LAST_TOP_IDX = None


def _build_program(nblk=NBLK, bch=BCH, group_w=GROUP_W):
    nc = bacc.Bacc(
        "TRN2",
        target_bir_lowering=False,
        debug=False,
        enable_asserts=False,
        num_devices=NCORES,
    )
    f32 = mybir.dt.float32
    f16 = mybir.dt.float16
    u32 = mybir.dt.uint32
    ns = nblk * BLK
    cand = nblk * TOPB
    b = bch * 128

    mT = nc.dram_tensor("mT", [D, ns], f16, kind="ExternalInput").ap()
    qT = nc.dram_tensor("qT", [D, b], f16, kind="ExternalInput").ap()
    msq = nc.dram_tensor("msq", [1, ns], f32, kind="ExternalInput").ap()
    out_vals = nc.dram_tensor("out_vals", [b, cand], f16, kind="ExternalOutput").ap()
    out_idx = nc.dram_tensor("out_idx", [b, cand], u32, kind="ExternalOutput").ap()

    mT_r = mT.rearrange("(o p) n -> p o n", p=128)
    qT_r = qT.rearrange("(o p) b -> p o b", p=128)
    ov_r = out_vals.rearrange("(c p) j -> p c j", p=128)
    oi_r = out_idx.rearrange("(c p) j -> p c j", p=128)

    groups = []
    g0 = 0
    while g0 < nblk:
        w = min(group_w, nblk - g0)
        groups.append((g0, w))
        g0 += w

    with tile.TileContext(nc) as tc:
        with (
            tc.tile_pool(name="const", bufs=1) as cpool,
            tc.tile_pool(name="mov", bufs=2) as movpool,
            tc.tile_pool(name="msqp", bufs=2) as msqpool,
            tc.tile_pool(name="score", bufs=6) as scpool,
            tc.tile_pool(name="psum", bufs=8, space="PSUM") as pspool,
        ):
            qt_sb = cpool.tile([128, KT, b], f16, tag="qt")
            nc.sync.dma_start(qt_sb, qT_r)
            cand_v = cpool.tile([128, bch, cand], f16, tag="cv")
            cand_i = cpool.tile([128, bch, cand], u32, tag="ci")

            for blk0, w in groups:
                n0 = blk0 * BLK
                wn = w * BLK
                mov = movpool.tile([128, KT, group_w * BLK], f16, tag="mov")
                nc.sync.dma_start(mov[:, :, :wn], mT_r[:, :, n0 : n0 + wn])
                msqb = msqpool.tile([128, group_w * BLK], f32, tag="msqb")
                nc.sync.dma_start(
                    msqb[:, :wn], msq[:, n0 : n0 + wn].to_broadcast([128, wn])
                )
                for bc in range(bch):
                    psums = [
                        pspool.tile([128, BLK], f32, tag="mm", name="mm_ps")
                        for _ in range(w)
                    ]
                    for k in range(KT):
                        lhsT = qt_sb[:, k, bc * 128 : (bc + 1) * 128]
                        for j in range(w):
                            nc.tensor.matmul(
                                psums[j],
                                lhsT=lhsT,
                                rhs=mov[:, k, j * BLK : (j + 1) * BLK],
                                start=(k == 0),
                                stop=(k == KT - 1),
                            )
                    for j in range(w):
                        blk = blk0 + j
                        sc = scpool.tile([128, BLK], f16, tag="sc", name="sc")
                        nc.vector.tensor_sub(
                            sc, psums[j], msqb[:, j * BLK : (j + 1) * BLK]
                        )
                        nc.vector.max(
                            out=cand_v[:, bc, blk * TOPB : (blk + 1) * TOPB], in_=sc
                        )
                        nc.vector.max_index(
                            out=cand_i[:, bc, blk * TOPB : (blk + 1) * TOPB],
                            in_max=cand_v[:, bc, blk * TOPB : (blk + 1) * TOPB],
                            in_values=sc,
                        )
            nc.sync.dma_start(ov_r, cand_v)
            nc.sync.dma_start(oi_r, cand_i)
    nc.compile()
    return nc


def kernel(h_query, memory_embeds, pred_values):
    global LAST_RUN, LAST_TOP_IDX
    q = np.ascontiguousarray(np.asarray(h_query, dtype=np.float32))
    m = np.ascontiguousarray(np.asarray(memory_embeds, dtype=np.float32))
    pv = np.asarray(pred_values, dtype=np.float32)

    qT16 = (np.ascontiguousarray(q.T) * np.float32(2.0)).astype(np.float16)
    msq = np.einsum("nd,nd->n", m, m)
    mT16 = np.ascontiguousarray(m.T).astype(np.float16)

    if "nc" not in _CACHE:
        _CACHE["nc"] = _build_program()
    nc = _CACHE["nc"]

    in_maps = []
    for c in range(NCORES):
        sl = slice(c * NS, (c + 1) * NS)
        in_maps.append(
            {
                "mT": np.ascontiguousarray(mT16[:, sl]),
                "qT": qT16,
                "msq": np.ascontiguousarray(msq[sl]).reshape(1, NS),
            }
        )

    res = bass_utils.run_bass_kernel_spmd(nc, in_maps, core_ids=list(range(NCORES)))
    LAST_RUN = res
    results = res.results

    vals = np.concatenate(
        [r["out_vals"].astype(np.float32) for r in results], axis=1
    )
    blk_off = (np.arange(CAND, dtype=np.int64) // TOPB) * BLK
    idxs = np.concatenate(
        [
            r["out_idx"].astype(np.int64) + blk_off[None, :] + c * NS
            for c, r in enumerate(results)
        ],
        axis=1,
    )

    # Phase 2: exact fp64 re-rank of the top-RERANK approximate candidates.
    sel = np.argpartition(-vals, RERANK, axis=1)[:, :RERANK]
    cidx = np.take_along_axis(idxs, sel, axis=1)           # [B, RERANK]
    mg = m[cidx].astype(np.float64)                        # [B, RERANK, D]
    s_exact = 2.0 * np.einsum("bd,bkd->bk", q.astype(np.float64), mg)
    s_exact -= np.einsum("bkd,bkd->bk", mg, mg)
    pick = np.argpartition(-s_exact, K, axis=1)[:, :K]
    top_idx = np.take_along_axis(cidx, pick, axis=1)
    LAST_TOP_IDX = top_idx
    y = pv[top_idx].astype(np.float64).mean()
    return np.float32(y)


# revision 11
# speedup vs baseline: 5.7604x; 1.4264x over previous
"""Distributed brute-force KNN (IndexFlatL2, K=3) + mean of gathered pred values.

Strategy (data-parallel over the memory bank N, queries replicated):
  - Each of the 8 cores gets a shard of 12500 memory rows (transposed on host
    so the PE moving operand [K=d, N=n] streams directly from DRAM).
  - Phase 1 (device): approximate scores s'[b, n] = 2*q.m_n - ||m_n||^2
    (largest s' == smallest L2 distance; q_sq is row-constant and drops out).
    fp8e4m3 DoubleRow matmuls (0.5 PE cycles/column, contraction pairs of
    k-subtiles), fp32 PSUM accumulation, msq subtracted exactly on DVE during
    the PSUM drain (fp16 scores). DVE pool_max windows the scores 10:1; one
    max8 + max_index over the 1250 window maxima per query chunk returns each
    query's top-8 windows per core. Approximate score noise (~1.5) is far
    below the capture margins (the global top-3 are extreme order statistics,
    ~100 above their block's rank-8 cutoff), so the true top-3 rows always
    survive inside the returned windows.
  - Phase 2 (host): rank the 64 windows per query by window max, take the
    top WSEL, exactly re-score their rows (fp64) and take the true top-3,
    gather pred_values, return the scalar mean.
"""

import sys
import types

import ml_dtypes
import numpy as np

try:  # bass_utils' axon trace path imports this unconditionally when
    import antenv.axon_hooks  # noqa: F401  # BASS_TRACE is set; stub it if absent
except ImportError:
    _stub = types.ModuleType("antenv.axon_hooks")
    _stub.get_axon_ntff_profile_hook = lambda: None
    _stub.set_axon_ntff_profile_hook = lambda hook: None
    sys.modules["antenv.axon_hooks"] = _stub

import concourse.bacc as bacc
import concourse.mybir as mybir
import concourse.tile as tile
from concourse import bass_utils

B = 1024            # queries
D = 1024            # embedding dim
N = 100000          # memory rows
NCORES = 8
NS = N // NCORES    # 12500 memory rows per core
BLK = 500           # matmul free-dim tile (fits one PSUM bank in fp32)
NBLK = NS // BLK    # 25 blocks per core
GROUP_W = 5         # blocks fetched per DMA group (25 = 5 uniform groups)
KT = D // 128       # 8 contraction tiles
BCH = B // 128      # 8 query chunks of 128
WND = 10            # pool_max window width
NWIN = NS // WND    # 1250 windows per core
TOPB = 8            # DVE max8 width -> top-8 windows per query per core
K = 3
WSEL = 10           # windows exactly re-scored on host per query

USE_FP8 = True      # False falls back to fp16 matmuls (k-step 1)

_CACHE = {}
LAST_RUN = None
LAST_TOP_IDX = None


def _build_program(nblk=NBLK, bch=BCH, group_w=GROUP_W):
    nc = bacc.Bacc(
        "TRN2",
        target_bir_lowering=False,
        debug=False,
        enable_asserts=False,
        num_devices=NCORES,
    )
    f32 = mybir.dt.float32
    f16 = mybir.dt.float16
    u32 = mybir.dt.uint32
    mmdt = mybir.dt.float8e4 if USE_FP8 else f16
    kstep = 2 if USE_FP8 else 1
    perf_mode = mybir.MatmulPerfMode.DoubleRow if USE_FP8 else None
    ns = nblk * BLK
    nwin = ns // WND
    wpb = BLK // WND  # windows per block
    b = bch * 128

    mT = nc.dram_tensor("mT", [D, ns], mmdt, kind="ExternalInput").ap()
    qT = nc.dram_tensor("qT", [D, b], mmdt, kind="ExternalInput").ap()
    msq = nc.dram_tensor("msq", [1, ns], f32, kind="ExternalInput").ap()
    out_vals = nc.dram_tensor("out_vals", [b, TOPB], f16, kind="ExternalOutput").ap()
    out_idx = nc.dram_tensor("out_idx", [b, TOPB], u32, kind="ExternalOutput").ap()

    mT_r = mT.rearrange("(o p) n -> p o n", p=128)
    qT_r = qT.rearrange("(o p) b -> p o b", p=128)
    ov_r = out_vals.rearrange("(c p) j -> p c j", p=128)
    oi_r = out_idx.rearrange("(c p) j -> p c j", p=128)

    groups = []
    g0 = 0
    while g0 < nblk:
        w = min(group_w, nblk - g0)
        groups.append((g0, w))
        g0 += w

    with tile.TileContext(nc) as tc:
        with (
            tc.tile_pool(name="const", bufs=1) as cpool,
            tc.tile_pool(name="mov", bufs=2) as movpool,
            tc.tile_pool(name="msqp", bufs=2) as msqpool,
            tc.tile_pool(name="score", bufs=6) as scpool,
            tc.tile_pool(name="psum", bufs=8, space="PSUM") as pspool,
        ):
            qt_sb = cpool.tile([128, KT, b], mmdt, tag="qt")
            nc.sync.dma_start(qt_sb, qT_r)
            wmax = cpool.tile([128, bch, nwin], f16, tag="wmax")
            cand_v = cpool.tile([128, bch, TOPB], f16, tag="cv")
            cand_i = cpool.tile([128, bch, TOPB], u32, tag="ci")

            for blk0, w in groups:
                n0 = blk0 * BLK
                wn = w * BLK
                mov = movpool.tile([128, KT, group_w * BLK], mmdt, tag="mov")
                nc.sync.dma_start(mov[:, :, :wn], mT_r[:, :, n0 : n0 + wn])
                msqb = msqpool.tile([128, group_w * BLK], f32, tag="msqb")
                nc.sync.dma_start(
                    msqb[:, :wn], msq[:, n0 : n0 + wn].to_broadcast([128, wn])
                )
                for bc in range(bch):
                    psums = [
                        pspool.tile([128, BLK], f32, tag="mm", name="mm_ps")
                        for _ in range(w)
                    ]
                    for k in range(0, KT, kstep):
                        lhsT = qt_sb[:, k : k + kstep, bc * 128 : (bc + 1) * 128]
                        for j in range(w):
                            nc.tensor.matmul(
                                psums[j],
                                lhsT=lhsT,
                                rhs=mov[:, k : k + kstep, j * BLK : (j + 1) * BLK],
                                start=(k == 0),
                                stop=(k + kstep >= KT),
                                perf_mode=perf_mode,
                            )
                    for j in range(w):
                        blk = blk0 + j
                        # window stride padded to WND+2 so the [w, t] dims
                        # can't merge at AP lowering (pool needs a real 3D AP)
                        sc = scpool.tile([128, wpb, WND + 2], f16, tag="sc", name="sc")
                        nc.vector.tensor_sub(
                            sc[:, :, :WND],
                            psums[j].rearrange("p (w t) -> p w t", t=WND),
                            msqb[:, j * BLK : (j + 1) * BLK].rearrange(
                                "p (w t) -> p w t", t=WND
                            ),
                        )
                        nc.vector.tensor_reduce(
                            wmax[:, bc, blk * wpb : (blk + 1) * wpb],
                            sc[:, :, :WND],
                            axis=mybir.AxisListType.X,
                            op=mybir.AluOpType.max,
                        )
            for bc in range(bch):
                nc.vector.max(out=cand_v[:, bc, :], in_=wmax[:, bc, :])
                nc.vector.max_index(
                    out=cand_i[:, bc, :],
                    in_max=cand_v[:, bc, :],
                    in_values=wmax[:, bc, :],
                )
            nc.sync.dma_start(ov_r, cand_v)
            nc.sync.dma_start(oi_r, cand_i)
    nc.compile()
    return nc


def kernel(h_query, memory_embeds, pred_values):
    global LAST_RUN, LAST_TOP_IDX
    q = np.ascontiguousarray(np.asarray(h_query, dtype=np.float32))
    m = np.ascontiguousarray(np.asarray(memory_embeds, dtype=np.float32))
    pv = np.asarray(pred_values, dtype=np.float32)

    mmdt_np = ml_dtypes.float8_e4m3 if USE_FP8 else np.float16
    qTs = (np.ascontiguousarray(q.T) * np.float32(2.0)).astype(mmdt_np)
    msq = np.einsum("nd,nd->n", m, m)
    mTs = np.ascontiguousarray(m.T).astype(mmdt_np)

    if "nc" not in _CACHE:
        _CACHE["nc"] = _build_program()
    nc = _CACHE["nc"]

    in_maps = []
    for c in range(NCORES):
        sl = slice(c * NS, (c + 1) * NS)
        in_maps.append(
            {
                "mT": np.ascontiguousarray(mTs[:, sl]),
                "qT": qTs,
                "msq": np.ascontiguousarray(msq[sl]).reshape(1, NS),
            }
        )

    res = bass_utils.run_bass_kernel_spmd(nc, in_maps, core_ids=list(range(NCORES)))
    LAST_RUN = res
    results = res.results

    # windows: value [B, 8] + window index [B, 8] per core; global window id
    # = core * NWIN + widx; window w covers rows [core*NS + w*WND, +WND).
    vals = np.concatenate(
        [r["out_vals"].astype(np.float32) for r in results], axis=1
    )  # [B, 8*NCORES]
    widx = np.concatenate(
        [r["out_idx"].astype(np.int64) + c * NWIN for c, r in enumerate(results)],
        axis=1,
    )

    # Phase 2: pick top-WSEL windows per query, exactly re-score their rows.
    sel = np.argpartition(-vals, WSEL, axis=1)[:, :WSEL]
    wsel = np.take_along_axis(widx, sel, axis=1)           # [B, WSEL]
    rows = wsel[:, :, None] * WND + np.arange(WND)[None, None, :]
    cidx = rows.reshape(B, WSEL * WND)                     # [B, WSEL*WND]
    mg = m[cidx].astype(np.float64)                        # [B, WSEL*WND, D]
    s_exact = 2.0 * np.einsum("bd,bkd->bk", q.astype(np.float64), mg)
    s_exact -= np.einsum("bkd,bkd->bk", mg, mg)
    pick = np.argpartition(-s_exact, K, axis=1)[:, :K]
    top_idx = np.take_along_axis(cidx, pick, axis=1)
    LAST_TOP_IDX = top_idx
    y = pv[top_idx].astype(np.float64).mean()
    return np.float32(y)


# revision 16
# speedup vs baseline: 6.0615x; 1.0523x over previous
"""Distributed brute-force KNN (IndexFlatL2, K=3) + mean of gathered pred values.

Strategy (data-parallel over the memory bank N, queries replicated):
  - Host sorts the memory rows by ||m||^2 and shards the sorted bank across
    the 8 cores (12500 rows each), transposed so the PE moving operand
    [K=d, N=n] streams straight from DRAM.
  - Phase 1 (device): c[b, n] = (2q).m_n via fp8e4m3 DoubleRow matmuls
    (0.5 PE cycles/column, contraction pairs of k-subtiles) into fp32 PSUM.
    DVE tensor_reduce window-maxes each PSUM block directly (windows of 10
    columns); because rows are msq-sorted, ||m||^2 is constant to ~0.05
    within a window, so the window's best score s' = 2q.m - ||m||^2 is
    recovered as wmax(c) - msq_window (one [128, 1250] subtract per query
    chunk). One max8 + max_index over the corrected window scores returns
    each query's top-8 windows per core. Approximate noise (fp8 ~1.5 +
    window msq spread ~0.05) is far below the capture margins (~100), so
    the true top-3 rows always survive inside the returned windows.
  - Phase 2 (host): rank the 64 windows per query, take the top WSEL,
    exactly re-score their rows (fp64), take the true top-3, gather
    pred_values (through the sort permutation), return the scalar mean.
"""

import sys
import types

import ml_dtypes
import numpy as np

try:  # bass_utils' axon trace path imports this unconditionally when
    import antenv.axon_hooks  # noqa: F401  # BASS_TRACE is set; stub it if absent
except ImportError:
    _stub = types.ModuleType("antenv.axon_hooks")
    _stub.get_axon_ntff_profile_hook = lambda: None
    _stub.set_axon_ntff_profile_hook = lambda hook: None
    sys.modules["antenv.axon_hooks"] = _stub

import concourse.bacc as bacc
import concourse.mybir as mybir
import concourse.tile as tile
from concourse import bass_utils

B = 1024            # queries
D = 1024            # embedding dim
N = 100000          # memory rows
NCORES = 8
NS = N // NCORES    # 12500 memory rows per core
BLK = 500           # matmul free-dim tile (fits one PSUM bank in fp32)
NBLK = NS // BLK    # 25 blocks per core
GROUP_W = 5         # blocks fetched per DMA group (25 = 5 uniform groups)
KT = D // 128       # 8 contraction tiles
BCH = B // 128      # 8 query chunks of 128
WND = 10            # window width for the DVE windowed max
NWIN = NS // WND    # 1250 windows per core
WPB = BLK // WND    # 50 windows per block
TOPB = 8            # DVE max8 width
NSEG = 25           # window segments per core; top-8 windows per segment
SEGW = NWIN // NSEG  # 250 windows per segment
NCAND = NSEG * TOPB  # 40 candidate windows per query per core
K = 3
WSEL = 32           # windows exactly re-scored on host per query

USE_FP8 = True      # False falls back to fp16 matmuls (k-step 1)

_CACHE = {}
LAST_RUN = None
LAST_TOP_IDX = None


def _build_program(nblk=NBLK, bch=BCH, group_w=GROUP_W):
    nc = bacc.Bacc(
        "TRN2",
        target_bir_lowering=False,
        debug=False,
        enable_asserts=False,
        num_devices=NCORES,
    )
    f32 = mybir.dt.float32
    u32 = mybir.dt.uint32
    mmdt = mybir.dt.float8e4 if USE_FP8 else mybir.dt.float16
    kstep = 2 if USE_FP8 else 1
    perf_mode = mybir.MatmulPerfMode.DoubleRow if USE_FP8 else None
    ns = nblk * BLK
    nwin = ns // WND
    b = bch * 128

    mT = nc.dram_tensor("mT", [D, ns], mmdt, kind="ExternalInput").ap()
    qT = nc.dram_tensor("qT", [D, b], mmdt, kind="ExternalInput").ap()
    msqw = nc.dram_tensor("msqw", [1, nwin], f32, kind="ExternalInput").ap()
    out_vals = nc.dram_tensor("out_vals", [b, NCAND], f32, kind="ExternalOutput").ap()
    out_idx = nc.dram_tensor("out_idx", [b, NCAND], u32, kind="ExternalOutput").ap()

    mT_r = mT.rearrange("(o p) n -> p o n", p=128)
    qT_r = qT.rearrange("(o p) b -> p o b", p=128)
    ov_r = out_vals.rearrange("(c p) j -> p c j", p=128)
    oi_r = out_idx.rearrange("(c p) j -> p c j", p=128)

    groups = []
    g0 = 0
    while g0 < nblk:
        w = min(group_w, nblk - g0)
        groups.append((g0, w))
        g0 += w

    with tile.TileContext(nc) as tc:
        with (
            tc.tile_pool(name="const", bufs=1) as cpool,
            tc.tile_pool(name="mov", bufs=2) as movpool,
            tc.tile_pool(name="wsc", bufs=3) as wscpool,
            tc.tile_pool(name="psum", bufs=8, space="PSUM") as pspool,
        ):
            qt_sb = cpool.tile([128, KT, b], mmdt, tag="qt")
            nc.sync.dma_start(qt_sb, qT_r)
            msqw_bc = cpool.tile([128, nwin], f32, tag="msqwbc")
            nc.sync.dma_start(msqw_bc, msqw.to_broadcast([128, nwin]))
            wmax = cpool.tile([128, bch, nwin], f32, tag="wmax")
            cand_v = cpool.tile([128, bch, NCAND], f32, tag="cv")
            cand_i = cpool.tile([128, bch, NCAND], u32, tag="ci")

            for blk0, w in groups:
                n0 = blk0 * BLK
                wn = w * BLK
                mov = movpool.tile([128, KT, group_w * BLK], mmdt, tag="mov")
                nc.sync.dma_start(mov[:, :, :wn], mT_r[:, :, n0 : n0 + wn])
                for bc in range(bch):
                    psums = [
                        pspool.tile([128, BLK], f32, tag="mm", name="mm_ps")
                        for _ in range(w)
                    ]
                    for k in range(0, KT, kstep):
                        lhsT = qt_sb[:, k : k + kstep, bc * 128 : (bc + 1) * 128]
                        for j in range(w):
                            nc.tensor.matmul(
                                psums[j],
                                lhsT=lhsT,
                                rhs=mov[:, k : k + kstep, j * BLK : (j + 1) * BLK],
                                start=(k == 0),
                                stop=(k + kstep >= KT),
                                perf_mode=perf_mode,
                            )
                    for j in range(w):
                        blk = blk0 + j
                        nc.vector.tensor_reduce(
                            wmax[:, bc, blk * WPB : (blk + 1) * WPB],
                            psums[j].rearrange("p (w t) -> p w t", t=WND),
                            axis=mybir.AxisListType.X,
                            op=mybir.AluOpType.max,
                            opt_input=False,
                        )
            segw = nwin // NSEG
            for bc in range(bch):
                wsc = wscpool.tile([128, nwin], f32, tag="wsc", name="wsc")
                nc.vector.tensor_sub(wsc, wmax[:, bc, :], msqw_bc)
                for f in range(NSEG):
                    seg = wsc[:, f * segw : (f + 1) * segw]
                    cv = cand_v[:, bc, f * TOPB : (f + 1) * TOPB]
                    nc.vector.max(out=cv, in_=seg)
                    nc.vector.max_index(
                        out=cand_i[:, bc, f * TOPB : (f + 1) * TOPB],
                        in_max=cv,
                        in_values=seg,
                    )
            nc.sync.dma_start(ov_r, cand_v)
            nc.sync.dma_start(oi_r, cand_i)
    nc.compile()
    return nc


def kernel(h_query, memory_embeds, pred_values):
    global LAST_RUN, LAST_TOP_IDX
    q = np.ascontiguousarray(np.asarray(h_query, dtype=np.float32))
    m = np.ascontiguousarray(np.asarray(memory_embeds, dtype=np.float32))
    pv = np.asarray(pred_values, dtype=np.float32)

    msq_full = np.einsum("nd,nd->n", m, m)
    perm = np.argsort(msq_full, kind="stable")
    m_s = m[perm]                      # msq-sorted memory bank
    msq_s = msq_full[perm]

    mmdt_np = ml_dtypes.float8_e4m3 if USE_FP8 else np.float16
    qTs = (np.ascontiguousarray(q.T) * np.float32(2.0)).astype(mmdt_np)
    mTs = np.ascontiguousarray(m_s.T).astype(mmdt_np)
    msqw_all = msq_s.reshape(N // WND, WND).mean(axis=1).astype(np.float32)

    if "nc" not in _CACHE:
        _CACHE["nc"] = _build_program()
    nc = _CACHE["nc"]

    in_maps = []
    for c in range(NCORES):
        sl = slice(c * NS, (c + 1) * NS)
        wsl = slice(c * NWIN, (c + 1) * NWIN)
        in_maps.append(
            {
                "mT": np.ascontiguousarray(mTs[:, sl]),
                "qT": qTs,
                "msqw": np.ascontiguousarray(msqw_all[wsl]).reshape(1, NWIN),
            }
        )

    res = bass_utils.run_bass_kernel_spmd(nc, in_maps, core_ids=list(range(NCORES)))
    LAST_RUN = res
    results = res.results

    # windows: value [B, 40] + in-segment index [B, 40] per core; global
    # window id = core*NWIN + seg*SEGW + idx; window w covers sorted rows
    # [w*WND, +WND).
    seg_off = (np.arange(NCAND, dtype=np.int64) // TOPB) * SEGW
    vals = np.concatenate([r["out_vals"] for r in results], axis=1)
    widx = np.concatenate(
        [
            r["out_idx"].astype(np.int64) + seg_off[None, :] + c * NWIN
            for c, r in enumerate(results)
        ],
        axis=1,
    )

    # Phase 2: pick top-WSEL windows per query, exactly re-score their rows.
    sel = np.argpartition(-vals, WSEL, axis=1)[:, :WSEL]
    wsel = np.take_along_axis(widx, sel, axis=1)           # [B, WSEL]
    rows = wsel[:, :, None] * WND + np.arange(WND)[None, None, :]
    cidx = rows.reshape(B, WSEL * WND)                     # sorted-space rows
    mg = m_s[cidx].astype(np.float64)                      # [B, WSEL*WND, D]
    s_exact = 2.0 * np.einsum("bd,bkd->bk", q.astype(np.float64), mg)
    s_exact -= np.einsum("bkd,bkd->bk", mg, mg)
    pick = np.argpartition(-s_exact, K, axis=1)[:, :K]
    top_sorted = np.take_along_axis(cidx, pick, axis=1)
    top_idx = perm[top_sorted]                             # original row ids
    LAST_TOP_IDX = top_idx
    y = pv[top_idx].astype(np.float64).mean()
    return np.float32(y)
